# revision 30
# baseline (speedup 1.0000x reference)
"""Local (7x7 window) attention kernel for Trainium2, 8 NeuronCores.

Problem: x[8,128,64,64]; q/k/v = 1x1-conv projections of x; attention over the
7x7 spatial neighborhood (zero-padded) summed over channels; softmax over the
49 window positions; y = attn-weighted sum of v over the window.

Sharding: data-parallel over batch B=8 -> one batch element per core.

Per-core layout strategy (C=128 lives on SBUF partitions):
  - x, q, k in [C, H*W] layout.  k is stored h-padded: [C, (H+6)*W] with 3
    zero rows above/below the image so every 2-query-row block sees a full
    8-key-row halo without branches.
  - v is produced TRANSPOSED, vt[pix, c], directly off the PE (x pixel-chunks
    as the stationary operand) so the attention-value matmul needs no
    on-chip transposes.
  - Scores are computed transposed: S^T[key, q] per 2-row query block
    (128 queries) as 4 chunk-matmuls (128 keys each, keys on PSUM
    partitions).  Softmax is max-free (scores are O(+-60), exp stays in f32
    range).  exp on ScalarE, band-mask multiply on VectorE against a
    host-precomputed [128,512] 0/1 mask (identical for every block).
  - AV: exp-scores chunks are the stationary operand, vt chunks stream ->
    yT[q, c] accumulated in PSUM.  Denominator d[q] via ones-vector matmuls
    accumulated in a separate PSUM bank + a host-precomputed additive
    correction for window positions that fall outside the image in the W
    direction (those contribute exp(0)=1 to the reference softmax).
  - normalize+bias fused in one VectorE scalar_tensor_tensor:
    y = (yT_psum * recip(d)) + bv_broadcast.

Output is written as yT [4096, 128] per core; the host restores [C,H,W].
"""

import sys

if "/opt/trn_rl_repo" not in sys.path:
    sys.path.insert(0, "/opt/trn_rl_repo")

import numpy as np

import concourse.bass as bass
import concourse.bacc as bacc
import concourse.mybir as mybir
from concourse import tile
from concourse.bass_utils import run_bass_kernel_spmd

F32 = mybir.dt.float32

B, C, H, W = 8, 128, 64, 64
KW = 7
PAD = KW // 2            # 3
HP = H + 2 * PAD         # 70 padded rows
NPIX = H * W             # 4096
NPPIX = HP * W           # 4480
RPB = 2                  # query rows per block
NBLK = H // RPB          # 32 blocks
NCHUNK = 4               # key chunks (of 128) per block
NVC = NPPIX // 128       # 35 vt chunks
GRP = 4                  # blocks per reciprocal batch

_CACHE = {}


def _build_mask():
    """maskT[p, 128*i + qi]: 1 if key (chunk i, within-chunk p) is inside the
    7x7 window of query qi, else 0.  Block-independent."""
    m = np.zeros((128, NCHUNK * 128), dtype=np.float32)
    for i in range(NCHUNK):
        for p in range(128):
            r, wk = p // 64, p % 64
            for qi in range(128):
                rq, wq = qi // 64, qi % 64
                dh = 2 * i + r - 3 - rq
                if abs(dh) <= PAD and abs(wk - wq) <= PAD:
                    m[p, 128 * i + qi] = 1.0
    return m


def _build_dx():
    """Denominator correction: #window positions outside the image in W,
    per query (7 window rows x missing w columns), as a [128,1] column
    (per-partition scalar for the pre-reciprocal add)."""
    dx = np.zeros((128, 1), dtype=np.float32)
    for qi in range(128):
        wq = qi % 64
        dx[qi, 0] = float(KW * (max(0, PAD - wq) + max(0, wq - (W - 1 - PAD))))
    return dx


def _build_bass():
    # Bacc (not plain Bass): its compile pipeline splits semaphore waits to
    # satisfy the TRN2 one-wait-per-instruction constraint
    # (move_matmul_waits_to_ldweights + generate_event_semaphores).
    nc = bacc.Bacc()

    x_d = nc.dram_tensor("x", [C, NPIX], F32, kind="ExternalInput")
    wqt_d = nc.dram_tensor("wqt", [C, C], F32, kind="ExternalInput")
    wkt_d = nc.dram_tensor("wkt", [C, C], F32, kind="ExternalInput")
    wvt_d = nc.dram_tensor("wvt", [C, C], F32, kind="ExternalInput")
    bq_d = nc.dram_tensor("bq", [C, 1], F32, kind="ExternalInput")
    bk_d = nc.dram_tensor("bk", [C, 1], F32, kind="ExternalInput")
    bvb_d = nc.dram_tensor("bvb", [128, C], F32, kind="ExternalInput")
    mask_d = nc.dram_tensor("maskT", [128, NCHUNK * 128], F32, kind="ExternalInput")
    dx_d = nc.dram_tensor("dxcol", [128, 1], F32, kind="ExternalInput")
    y_d = nc.dram_tensor("y", [NPIX, C], F32, kind="ExternalOutput")

    with tile.TileContext(nc) as tc:
        with (
            tc.tile_pool(name="const", bufs=1) as cpool,
            tc.tile_pool(name="big", bufs=1) as bigpool,
            tc.tile_pool(name="sb_exp", bufs=3) as sb_exp,
            tc.tile_pool(name="sb_expm", bufs=3) as sb_expm,
            tc.tile_pool(name="sb_y", bufs=4) as sb_y,
        ):
            # ---- constants / persistent tensors ----
            wqt = cpool.tile([C, C], F32)
            wkt = cpool.tile([C, C], F32)
            wvt = cpool.tile([C, C], F32)
            bq = cpool.tile([C, 1], F32)
            bk = cpool.tile([C, 1], F32)
            maskB = cpool.tile([128, NCHUNK * 128], F32)
            dxcol = cpool.tile([128, 1], F32)
            ones_col = cpool.tile([128, 1], F32)
            bvb = cpool.tile([128, C], F32)      # bv broadcast (host-provided)
            rd = cpool.tile([128, NBLK], F32)    # per-block reciprocal denoms

            x_s = bigpool.tile([C, NPIX], F32)
            q_s = bigpool.tile([C, NPIX], F32)
            kp_s = bigpool.tile([C, NPPIX], F32)
            vt_s = bigpool.tile([128, NVC * 128], F32)

            nc.sync.dma_start(wqt[:], wqt_d[:])
            nc.sync.dma_start(wkt[:], wkt_d[:])
            nc.sync.dma_start(wvt[:], wvt_d[:])
            nc.sync.dma_start(bq[:], bq_d[:])
            nc.sync.dma_start(bk[:], bk_d[:])
            nc.sync.dma_start(bvb[:], bvb_d[:])
            nc.sync.dma_start(maskB[:], mask_d[:])
            nc.sync.dma_start(dxcol[:], dx_d[:])
            for m in range(8):
                sl = slice(512 * m, 512 * (m + 1))
                nc.sync.dma_start(x_s[:, sl], x_d[:, sl])

            nc.gpsimd.memset(ones_col[:], 1.0)
            # zero-padding regions of kp and vt
            nc.gpsimd.memset(kp_s[:, 0 : PAD * W], 0.0)
            nc.gpsimd.memset(kp_s[:, (PAD + H) * W : NPPIX], 0.0)
            nc.gpsimd.memset(vt_s[:, 0 : 2 * 128], 0.0)
            nc.gpsimd.memset(vt_s[:, (NVC - 2) * 128 : NVC * 128], 0.0)

            # ---- projections ----
            with (
                tc.tile_pool(name="ps_qk", bufs=3, space="PSUM") as ps_qk,
                tc.tile_pool(name="ps_v", bufs=3, space="PSUM") as ps_v,
            ):
                for m in range(8):
                    sl = slice(512 * m, 512 * (m + 1))
                    pq = ps_qk.tile([128, 512], F32, tag="pqk", name=f"pq{m}")
                    nc.tensor.matmul(pq[:], wqt[:], x_s[:, sl], start=True, stop=True)
                    nc.vector.tensor_scalar_add(q_s[:, sl], pq[:], bq[:])
                for m in range(8):
                    sl = slice(512 * m, 512 * (m + 1))
                    ksl = slice(PAD * W + 512 * m, PAD * W + 512 * (m + 1))
                    pk = ps_qk.tile([128, 512], F32, tag="pqk", name=f"pk{m}")
                    nc.tensor.matmul(pk[:], wkt[:], x_s[:, sl], start=True, stop=True)
                    nc.vector.tensor_scalar_add(kp_s[:, ksl], pk[:], bk[:])
                # vt: chunk j covers padded rows (2j, 2j+1); image row h lives
                # at padded row h+3, so chunk j holds image rows (2j-3, 2j-2).
                for j in range(1, NVC - 1):
                    pv = ps_v.tile([128, C], F32, tag="pv", name=f"pv{j}")
                    if j == 1:
                        lhsT = x_s[:, 0:64]            # image row 0 -> prow 3
                        out = pv[64:128, :]
                        dst = vt_s[64:128, 128 * j : 128 * (j + 1)]
                    elif j == NVC - 2:
                        lhsT = x_s[:, (H - 1) * W : NPIX]  # row 63 -> prow 66
                        out = pv[0:64, :]
                        dst = vt_s[0:64, 128 * j : 128 * (j + 1)]
                    else:
                        r0 = 2 * j - 3
                        lhsT = x_s[:, r0 * W : (r0 + 2) * W]
                        out = pv[:, :]
                        dst = vt_s[:, 128 * j : 128 * (j + 1)]
                    nc.tensor.matmul(out, lhsT, wvt[:], start=True, stop=True)
                    nc.vector.tensor_copy(dst, out)

            # ---- attention blocks ----
            ps_s = tc.alloc_tile_pool(name="ps_s", bufs=3, space="PSUM")
            ps_av = tc.alloc_tile_pool(name="ps_av", bufs=2, space="PSUM")
            ps_d = tc.alloc_tile_pool(name="ps_d", bufs=1, space="PSUM")
            dps = ps_d.tile([128, NBLK], F32)
            expm_tiles = {}
            pav_tiles = {}

            def s_phase(b):
                sps = ps_s.tile([128, NCHUNK * 128], F32, tag="sps")
                for i in range(NCHUNK):
                    kc = W * (RPB * b + 2 * i)
                    nc.tensor.matmul(
                        sps[:, 128 * i : 128 * (i + 1)],
                        kp_s[:, kc : kc + 128],
                        q_s[:, 128 * b : 128 * (b + 1)],
                        start=True,
                        stop=True,
                    )
                # mask as additive -1e9 BEFORE exp: exp(-1e9)=0 exactly, so
                # out-of-window scores can never become inf*0=NaN on the HW
                # spline exp (observed: masked score >88 -> inf on ACT).
                sm = sb_exp.tile([128, NCHUNK * 128], F32, tag="sm")
                nc.vector.tensor_add(sm[:], sps[:], maskB[:])
                em = sb_expm.tile([128, NCHUNK * 128], F32, tag="em")
                nc.scalar.activation(em[:], sm[:], mybir.ActivationFunctionType.Exp)
                expm_tiles[b] = em

            def av_phase(b):
                em = expm_tiles.pop(b)
                g, bb = b // GRP, b % GRP
                if bb == 0:
                    pav_tiles[g] = ps_av.tile(
                        [128, GRP * 128], F32, tag="pav", name=f"pav{g}"
                    )
                pav = pav_tiles[g][:, 128 * bb : 128 * (bb + 1)]
                for i in range(NCHUNK):
                    vc = 128 * (b + i)
                    nc.tensor.matmul(
                        pav,
                        em[:, 128 * i : 128 * (i + 1)],
                        vt_s[:, vc : vc + 128],
                        start=(i == 0),
                        stop=(i == NCHUNK - 1),
                    )
                for i in range(NCHUNK):
                    nc.tensor.matmul(
                        dps[:, b : b + 1],
                        em[:, 128 * i : 128 * (i + 1)],
                        ones_col[:],
                        start=(i == 0),
                        stop=(i == NCHUNK - 1),
                    )

            def norm_phase(g):
                b0 = GRP * g
                dsum = sb_y.tile([128, GRP], F32, tag="dsum", name=f"dsum{g}")
                nc.vector.tensor_scalar_add(dsum[:], dps[:, b0 : b0 + GRP], dxcol[:])
                nc.vector.reciprocal(rd[:, b0 : b0 + GRP], dsum[:])
                pav = pav_tiles.pop(g)
                for bb in range(GRP):
                    b = b0 + bb
                    ysb = sb_y.tile([128, C], F32, tag="ysb")
                    nc.vector.scalar_tensor_tensor(
                        ysb[:],
                        pav[:, 128 * bb : 128 * (bb + 1)],
                        rd[:, b : b + 1],
                        bvb[:],
                        op0=mybir.AluOpType.mult,
                        op1=mybir.AluOpType.add,
                    )
                    nc.sync.dma_start(y_d[128 * b : 128 * (b + 1), :], ysb[:])

            # software-pipelined emission so the in-order PE never stalls on
            # the ACT/DVE exp+mask of the same block
            for b in range(NBLK):
                s_phase(b)
                if b >= 1:
                    av_phase(b - 1)
                if b >= 1 and (b - 1) % GRP == GRP - 1:
                    norm_phase((b - 1) // GRP)
            av_phase(NBLK - 1)
            norm_phase((NBLK - 1) // GRP)
            ps_d.release()
            ps_av.release()
            ps_s.release()

    nc.finalize()
    return nc


def get_nc():
    if "nc" not in _CACHE:
        _CACHE["nc"] = _build_bass()
    return _CACHE["nc"]


def prepare_in_maps(x, Wq, bq, Wk, bk, Wv, bv):
    x = np.ascontiguousarray(np.asarray(x, dtype=np.float32))
    if "maskb" not in _CACHE:
        _CACHE["maskb"] = np.ascontiguousarray((_build_mask() - 1.0) * 1e9)
        _CACHE["dx"] = _build_dx()
    common = {
        "wqt": np.ascontiguousarray(np.asarray(Wq, np.float32).T),
        "wkt": np.ascontiguousarray(np.asarray(Wk, np.float32).T),
        "wvt": np.ascontiguousarray(np.asarray(Wv, np.float32).T),
        "bq": np.asarray(bq, np.float32).reshape(C, 1),
        "bk": np.asarray(bk, np.float32).reshape(C, 1),
        "bvb": np.ascontiguousarray(
            np.tile(np.asarray(bv, np.float32).reshape(1, C), (128, 1))
        ),
        "maskT": _CACHE["maskb"],
        "dxcol": _CACHE["dx"],
    }
    return [dict(common, x=x[b].reshape(C, NPIX)) for b in range(B)]


def gather_output(results):
    yt = np.stack([results[b]["y"] for b in range(B)])  # [B, 4096, 128]
    return np.ascontiguousarray(yt.transpose(0, 2, 1).reshape(B, C, H, W))


def kernel(x, Wq, bq, Wk, bk, Wv, bv):
    in_maps = prepare_in_maps(x, Wq, bq, Wk, bk, Wv, bv)
    res = run_bass_kernel_spmd(get_nc(), in_maps, list(range(B))).results
    return gather_output(res)


if __name__ == "__main__":
    rng = np.random.default_rng(0)
    xs = rng.standard_normal((B, C, H, W), dtype=np.float32)
    ws = [rng.standard_normal((C, C), dtype=np.float32) / np.sqrt(C) for _ in range(3)]
    bs = [rng.standard_normal(C).astype(np.float32) * 0.01 for _ in range(3)]
    y = kernel(xs, ws[0], bs[0], ws[1], bs[1], ws[2], bs[2])
    print(y.shape, y.dtype)
